# revision 6
# baseline (speedup 1.0000x reference)
"""Causal single-head attention (B=4, S=2048, d=1024, f32) on 8 TRN2 NeuronCores.

v3: score factorization + pairwise V exchange. Core i = (b = i//2, h = i%2).

The K projection is eliminated algebraically: scores = (x Wq)(x Wk)^T =
x (Wq Wk^T) x^T, so the host passes M = Wq Wk^T (computed once in fp32, cast
bf16) and the kernel computes Q' = x_q M for its own queries (same cost as the
old Q projection) and contracts scores directly against the resident x^T input
tile — the stationary operand of the score matmuls is xkv itself. This removes
~27us of PE work per core AND the K-half exchange of v2. Numerically this is
still a depth-2 product chain of 1024-deep bf16 dots, so accuracy matches the
two-projection form.

The V projection remains deduplicated as in v2: each core computes V columns
h*512..h*512+511 into staging columns 0-511, pairwise-AllGathers the halves
(groups [[0,1],[2,3],[4,5],[6,7]]) via DRAM bounce buffers in two 1MB strips,
and downloads both gather blocks into global-ordered columns (uniform SPMD
program; only the host-side wv input is a per-core column slice of Wv).

V proj runs first so the exchange is issued ~1/8 into the rep; Q' proj is
qf-outer so early attention chunks unblock first. xkv is double-buffered so
the next rep's loads overlap this rep's attention (scores read xkv to the end).

Everything else (zig-zag 256-row query blocks, transposed scores P^T = exp(S^T)
feeding AV as lhsT, ones-column denominator matmul folded into the PSUM->SBUF
copy, bf16 compute with f32 PSUM accumulation, DMA "blessing" via in-place DVE
copies, additive causal masks from host) matches v1/v2.
"""

import numpy as np
import ml_dtypes

import concourse.bass as bass
from concourse import bacc
import concourse.mybir as mybir
from concourse.tile import TileContext
from concourse.bass_utils import run_bass_kernel_spmd

P = 128
B = 4
S = 2048          # sequence length (= keys per batch)
D = 1024          # d_in = d_out
HALF = 1024       # queries per core
CHUNK = 256       # query chunk
CD = D // P       # 8 contraction tiles
SK = S // P       # 16 key tiles
F = 512           # matmul moving free dim (one PSUM bank of f32)
VSTRIP = 8        # V exchange strip height (key tiles per strip)
KV = (512, 1024, 1536, 2048)   # kv prefix length per chunk
QBASE = (0, 256, 512, 768)     # local query offset of each chunk
SCALE = 1.0 / 32.0    # 1/sqrt(d_k)
NEG = -1.0e30         # additive mask for disallowed (k, q)

# global query-row block starts per half (zig-zag over 256-blocks)
QROWS = ((0, 512, 1280, 1792), (256, 768, 1024, 1536))
GROUPS = [[0, 1], [2, 3], [4, 5], [6, 7]]

BF16 = ml_dtypes.bfloat16


def build_nc(reps: int = 1, mock_cc: bool = False) -> bacc.Bacc:
    nc = bacc.Bacc("TRN2")
    bf = mybir.dt.bfloat16
    f32 = mybir.dt.float32

    def exchange(up, gather):
        if mock_cc:
            nc.sync.dma_start(out=gather[0], in_=up[:])
            nc.sync.dma_start(out=gather[1], in_=up[:])
        else:
            nc.gpsimd.collective_compute(
                "AllGather", mybir.AluOpType.bypass,
                replica_groups=GROUPS,
                ins=[up[:].opt()], outs=[gather[:].opt()],
            )

    xkv_d = nc.declare_dram_parameter("xkv", [D, S], bf, isOutput=False)
    xq_d = nc.declare_dram_parameter("xq", [D, HALF], bf, isOutput=False)
    wqk_d = nc.declare_dram_parameter("wqk", [D, D], bf, isOutput=False)
    wv_d = nc.declare_dram_parameter("wv", [D, F], bf, isOutput=False)
    m_d = [
        nc.declare_dram_parameter(f"mask{ci}", [KV[ci], CHUNK], bf, isOutput=False)
        for ci in range(len(KV))
    ]
    out_d = nc.declare_dram_parameter("out", [HALF, D], bf, isOutput=True)

    with TileContext(nc) as tc:
        with tc.tile_pool(name="persist", bufs=1) as persist, \
             tc.tile_pool(name="work", bufs=1) as work, \
             tc.tile_pool(name="dram", bufs=1, space="DRAM") as dram, \
             tc.tile_pool(name="psum", bufs=1, space="PSUM") as psum:
            # V[k, d], Q'^T[d, q] resident in SBUF (bf16)
            Vt = persist.tile([P, SK, D], bf)
            QT = persist.tile([P, CD, HALF], bf)
            ones = persist.tile([P, 1], bf)
            nc.vector.memset(ones[:], 1.0)

            def load(dst, dram_t, c):
                nc.sync.dma_start(out=dst[:, c], in_=dram_t[c * P:(c + 1) * P, :])
                nc.vector.tensor_copy(dst[:, c], dst[:, c])

            for _rep in range(reps):
                # xkv double-buffered: scores read it until the end of the
                # rep, the next rep's loads must overlap this rep's attention
                xkv_s = work.tile([P, CD, S], bf, tag="big", bufs=2)
                xq_s = work.tile([P, CD, HALF], bf, tag="xq")
                wqk_s = work.tile([P, CD, D], bf, tag="wqk")
                wv_s = work.tile([P, CD, F], bf, tag="wv")
                # wv/xkv first: the V matmuls depend only on these
                for c in range(CD):
                    load(wv_s, wv_d, c)
                    load(xkv_s, xkv_d, c)
                for c in range(CD):
                    load(wqk_s, wqk_d, c)
                    load(xq_s, xq_d, c)

                vup = [dram.tile([P, VSTRIP, F], bf, name=f"vup{i}",
                                 tag=f"vup{i}", bufs=2)
                       for i in range(SK // VSTRIP)]
                vg = [dram.tile([2, P, VSTRIP, F], bf, name=f"vg{i}",
                                tag=f"vg{i}", bufs=2)
                      for i in range(SK // VSTRIP)]

                # ---------------- phase 1: projections ----------------
                # V own half -> staging columns 0-511; exchange per strip
                for vs in range(SK // VSTRIP):
                    for sti in range(VSTRIP):
                        st = vs * VSTRIP + sti
                        ps = psum.tile([P, F], f32, tag="pp", bufs=2)
                        for c in range(CD):
                            nc.tensor.matmul(
                                ps[:],
                                xkv_s[:, c, st * P:(st + 1) * P],
                                wv_s[:, c, 0:F],
                                start=(c == 0), stop=(c == CD - 1),
                            )
                        nc.vector.tensor_copy(Vt[:, st, 0:F], ps[:])
                    nc.sync.dma_start(
                        out=vup[vs][:],
                        in_=Vt[:, vs * VSTRIP:(vs + 1) * VSTRIP, 0:F])
                    exchange(vup[vs], vg[vs])
                # Q'^T = M^T xq^T (global m order, qf-outer)
                for qf in range(HALF // F):
                    for m in range(CD):
                        ps = psum.tile([P, F], f32, tag="pp", bufs=2)
                        for c in range(CD):
                            nc.tensor.matmul(
                                ps[:],
                                wqk_s[:, c, m * P:(m + 1) * P],
                                xq_s[:, c, qf * F:(qf + 1) * F],
                                start=(c == 0), stop=(c == CD - 1),
                            )
                        nc.vector.tensor_copy(QT[:, m, qf * F:(qf + 1) * F], ps[:])

                # V exchange download: block j -> global columns, then bless
                for vs in range(SK // VSTRIP):
                    for j in range(2):
                        nc.sync.dma_start(
                            out=Vt[:, vs * VSTRIP:(vs + 1) * VSTRIP,
                                   j * F:(j + 1) * F],
                            in_=vg[vs][j])
                        for sti in range(VSTRIP):
                            st = vs * VSTRIP + sti
                            nc.vector.tensor_copy(
                                Vt[:, st, j * F:(j + 1) * F],
                                Vt[:, st, j * F:(j + 1) * F])

                # ---------------- phase 2: attention ----------------
                # scores^T[k, q] = sum_c x^T[c, k]^T Q'^T[c, q]: the resident
                # xkv tile IS the stationary operand (no K tensor exists)
                for ci in range(len(KV)):
                    nk = KV[ci] // P
                    qb = QBASE[ci]
                    md = m_d[ci]
                    PT = work.tile([P, SK, CHUNK], bf, tag="pt")
                    vmin = min(QROWS[0][ci], QROWS[1][ci])
                    for ki in range(nk):
                        masked = ki * P + P - 1 > vmin
                        if masked:
                            mt = work.tile([P, CHUNK], bf, tag="mask", bufs=4)
                            nc.sync.dma_start(
                                out=mt[:], in_=md[ki * P:(ki + 1) * P, :])
                            nc.vector.tensor_copy(mt[:], mt[:])
                        ps = psum.tile([P, CHUNK], f32, tag="pp", bufs=2)
                        for c in range(CD):
                            nc.tensor.matmul(
                                ps[:],
                                xkv_s[:, c, ki * P:(ki + 1) * P],
                                QT[:, c, qb:qb + CHUNK],
                                start=(c == 0), stop=(c == CD - 1),
                            )
                        if masked:
                            nc.vector.tensor_add(ps[:], ps[:], mt[:])
                        pe = work.tile([P, CHUNK], bf, tag="pexp", bufs=2)
                        nc.scalar.activation(
                            pe[:], ps[:],
                            mybir.ActivationFunctionType.Exp, scale=SCALE,
                        )
                        nc.vector.tensor_copy(PT[:, ki], pe[:])
                    for qj in range(CHUNK // P):
                        o0 = psum.tile([P, F], f32, tag="av", bufs=4)
                        o1 = psum.tile([P, F], f32, tag="av", bufs=4)
                        rs = psum.tile([P, 1], f32, tag="rs", bufs=2)
                        for ki in range(nk):
                            lh = PT[:, ki, qj * P:(qj + 1) * P]
                            st_, sp_ = (ki == 0), (ki == nk - 1)
                            nc.tensor.matmul(o0[:], lh, Vt[:, ki, 0:F],
                                             start=st_, stop=sp_)
                            nc.tensor.matmul(o1[:], lh, Vt[:, ki, F:2 * F],
                                             start=st_, stop=sp_)
                            nc.tensor.matmul(rs[:], lh, ones[:, 0:1],
                                             start=st_, stop=sp_)
                        rcp = work.tile([P, 1], f32, tag="rcp", bufs=4)
                        nc.vector.reciprocal(rcp[:], rs[:])
                        ot = work.tile([P, D], bf, tag="ot", bufs=4)
                        nc.vector.tensor_scalar_mul(ot[:, 0:F], o0[:], rcp[:])
                        nc.vector.tensor_scalar_mul(ot[:, F:2 * F], o1[:], rcp[:])
                        row = qb + qj * P
                        nc.sync.dma_start(out=out_d[row:row + P, :], in_=ot[:])
    nc.finalize()
    return nc


_NC_CACHE = {}


def _get_nc(reps: int = 1):
    if reps not in _NC_CACHE:
        _NC_CACHE[reps] = build_nc(reps)
    return _NC_CACHE[reps]


def _masks():
    """Additive bf16 masks per half: 0 where k <= global q position, else -1e30."""
    q = np.arange(CHUNK)[None, :]
    out = []
    for h in range(2):
        ms = []
        for ci in range(len(KV)):
            k = np.arange(KV[ci])[:, None]
            ms.append(np.where(k <= q + QROWS[h][ci], 0.0, NEG).astype(BF16))
        out.append(ms)
    return out


def make_in_maps(x, Wq, Wk, Wv):
    M = np.asarray(Wq, np.float32) @ np.asarray(Wk, np.float32).T
    wqkb = np.ascontiguousarray(M.astype(BF16))
    wvb = np.asarray(Wv).astype(BF16)
    masks = _masks()
    in_maps = []
    for i in range(8):
        b, h = i // 2, i % 2
        xT = np.ascontiguousarray(x[b].T.astype(BF16))
        xq = np.concatenate([x[b, r:r + CHUNK] for r in QROWS[h]], axis=0)
        xqT = np.ascontiguousarray(xq.T.astype(BF16))
        m = {
            "xkv": xT, "xq": xqT, "wqk": wqkb,
            "wv": np.ascontiguousarray(wvb[:, h * F:(h + 1) * F]),
        }
        for ci in range(len(KV)):
            m[f"mask{ci}"] = masks[h][ci]
        in_maps.append(m)
    return in_maps


def gather_out(results, x_dtype=np.float32):
    out = np.empty((B, S, D), x_dtype)
    for i in range(8):
        b, h = i // 2, i % 2
        o = np.asarray(results[i]["out"]).astype(x_dtype)
        for ci, r in enumerate(QROWS[h]):
            out[b, r:r + CHUNK] = o[ci * CHUNK:(ci + 1) * CHUNK]
    return out


def run_cores(in_maps, **kwargs):
    return run_bass_kernel_spmd(_get_nc(), in_maps, core_ids=list(range(8)), **kwargs)


def kernel(x, Wq, Wk, Wv):
    x = np.asarray(x)
    in_maps = make_in_maps(x, np.asarray(Wq), np.asarray(Wk), np.asarray(Wv))
    res = run_cores(in_maps)
    return gather_out(res.results)


# revision 7
# speedup vs baseline: 1.0160x; 1.0160x over previous
"""Causal single-head attention (B=4, S=2048, d=1024, f32) on 8 TRN2 NeuronCores.

v4: fully factorized, collective-free. Core i = (b = i//2, h = i%2) with the
zig-zag 256-row query-block assignment of v1-v3.

Both weight projections are fused away algebraically:
  scores = (x Wq)(x Wk)^T = x (Wq Wk^T) x^T   -> host passes M = Wq Wk^T,
      the kernel computes Q' = x_q M (qf-outer) and contracts scores against
      the resident x^T tile (stationary = xkv itself; no K tensor exists).
  out = P V = (P x) Wv                        -> per chunk the kernel computes
      U^T = x^T-contracted P (stationary = untransposed x tiles, moving = P^T)
      and then out = U Wv with the full Wv resident in SBUF.
Total matmul cycles equal v3's, but there is NO V projection, NO pairwise
exchange, and NO collectives — every core works standalone on its (batch,
query-half), which removes the cross-core rendezvous from the critical path.

The denominator is unchanged: per (chunk, qj) a ones-column matmul accumulates
sum_k P^T[k, q] into PSUM alongside; the division is folded into the final
PSUM->SBUF copy (per-partition tensor_scalar_mul), valid because out[q, n]
lands q-partitioned exactly as in v3.

x is needed in both layouts (x^T for scores / Q', x for the P-contraction), so
the host passes xkv [d, S] and xun [S, d]; both are single-buffered — their
next-rep loads start after this rep's last stationary read, which leaves
enough slack to hide the 4MB reloads under the following rep's early phases.

bf16 compute with f32 PSUM accumulation, DMA blessing via in-place DVE copies,
host-precomputed additive causal masks — all as in v1-v3.
"""

import numpy as np
import ml_dtypes

import concourse.bass as bass
from concourse import bacc
import concourse.mybir as mybir
from concourse.tile import TileContext
from concourse.bass_utils import run_bass_kernel_spmd

P = 128
B = 4
S = 2048          # sequence length (= keys per batch)
D = 1024          # d_in = d_out
HALF = 1024       # queries per core
CHUNK = 256       # query chunk
CD = D // P       # 8 contraction tiles
SK = S // P       # 16 key tiles
F = 512           # matmul moving free dim (one PSUM bank of f32)
KV = (512, 1024, 1536, 2048)   # kv prefix length per chunk
QBASE = (0, 256, 512, 768)     # local query offset of each chunk
SCALE = 1.0 / 32.0    # 1/sqrt(d_k)
NEG = -1.0e30         # additive mask for disallowed (k, q)

QROWS = ((0, 512, 1280, 1792), (256, 768, 1024, 1536))

BF16 = ml_dtypes.bfloat16


def build_nc(reps: int = 1) -> bacc.Bacc:
    nc = bacc.Bacc("TRN2")
    bf = mybir.dt.bfloat16
    f32 = mybir.dt.float32

    xkv_d = nc.declare_dram_parameter("xkv", [D, S], bf, isOutput=False)
    xun_d = nc.declare_dram_parameter("xun", [S, D], bf, isOutput=False)
    xq_d = nc.declare_dram_parameter("xq", [D, HALF], bf, isOutput=False)
    wqk_d = nc.declare_dram_parameter("wqk", [D, D], bf, isOutput=False)
    wv_d = nc.declare_dram_parameter("wv", [D, D], bf, isOutput=False)
    m_d = [
        nc.declare_dram_parameter(f"mask{ci}", [KV[ci], CHUNK], bf, isOutput=False)
        for ci in range(len(KV))
    ]
    out_d = nc.declare_dram_parameter("out", [HALF, D], bf, isOutput=True)

    with TileContext(nc) as tc:
        with tc.tile_pool(name="persist", bufs=1) as persist, \
             tc.tile_pool(name="work", bufs=1) as work, \
             tc.tile_pool(name="psum", bufs=1, space="PSUM") as psum:
            QT = persist.tile([P, CD, HALF], bf)
            ones = persist.tile([P, 1], bf)
            nc.vector.memset(ones[:], 1.0)

            def load(dst, dram_t, c):
                nc.sync.dma_start(out=dst[:, c], in_=dram_t[c * P:(c + 1) * P, :])
                nc.vector.tensor_copy(dst[:, c], dst[:, c])

            for _rep in range(reps):
                xq_s = work.tile([P, CD, HALF], bf, tag="xq")
                wqk_s = work.tile([P, CD, D], bf, tag="wqk")
                xkv_s = work.tile([P, CD, S], bf, tag="big")
                xun_s = work.tile([P, SK, D], bf, tag="xun")
                wv_s = work.tile([P, CD, D], bf, tag="wv", bufs=2)
                # xq/wqk first: Q' proj is the first PE work of the rep
                for c in range(CD):
                    load(xq_s, xq_d, c)
                    load(wqk_s, wqk_d, c)
                for c in range(CD):
                    load(xkv_s, xkv_d, c)
                    load(wv_s, wv_d, c)
                for c in range(SK):
                    load(xun_s, xun_d, c)

                # ---------------- Q'^T = M^T xq^T (qf-outer) ----------------
                for qf in range(HALF // F):
                    for m in range(CD):
                        ps = psum.tile([P, F], f32, tag="pp", bufs=2)
                        for c in range(CD):
                            nc.tensor.matmul(
                                ps[:],
                                wqk_s[:, c, m * P:(m + 1) * P],
                                xq_s[:, c, qf * F:(qf + 1) * F],
                                start=(c == 0), stop=(c == CD - 1),
                            )
                        nc.vector.tensor_copy(QT[:, m, qf * F:(qf + 1) * F], ps[:])

                # ---------------- attention ----------------
                for ci in range(len(KV)):
                    nk = KV[ci] // P
                    qb = QBASE[ci]
                    md = m_d[ci]
                    PT = work.tile([P, SK, CHUNK], bf, tag="pt")
                    vmin = min(QROWS[0][ci], QROWS[1][ci])
                    # scores^T[k, q] = sum_c x^T[c, k]^T Q'^T[c, q]
                    for ki in range(nk):
                        masked = ki * P + P - 1 > vmin
                        if masked:
                            mt = work.tile([P, CHUNK], bf, tag="mask", bufs=4)
                            nc.sync.dma_start(
                                out=mt[:], in_=md[ki * P:(ki + 1) * P, :])
                            nc.vector.tensor_copy(mt[:], mt[:])
                        ps = psum.tile([P, CHUNK], f32, tag="pp", bufs=2)
                        for c in range(CD):
                            nc.tensor.matmul(
                                ps[:],
                                xkv_s[:, c, ki * P:(ki + 1) * P],
                                QT[:, c, qb:qb + CHUNK],
                                start=(c == 0), stop=(c == CD - 1),
                            )
                        if masked:
                            nc.vector.tensor_add(ps[:], ps[:], mt[:])
                        pe = work.tile([P, CHUNK], bf, tag="pexp", bufs=2)
                        nc.scalar.activation(
                            pe[:], ps[:],
                            mybir.ActivationFunctionType.Exp, scale=SCALE,
                        )
                        nc.vector.tensor_copy(PT[:, ki], pe[:])
                    # U^T[d, q] = sum_k x[k, d]^T P^T[k, q]  (both qj at once)
                    UT = work.tile([P, CD, CHUNK], bf, tag="ut", bufs=2)
                    for db in range(CD):
                        pu = psum.tile([P, CHUNK], f32, tag="pp", bufs=2)
                        for ki in range(nk):
                            nc.tensor.matmul(
                                pu[:],
                                xun_s[:, ki, db * P:(db + 1) * P],
                                PT[:, ki, 0:CHUNK],
                                start=(ki == 0), stop=(ki == nk - 1),
                            )
                        nc.vector.tensor_copy(UT[:, db, :], pu[:])
                    # out[q, n] = sum_d U^T[d, q]^T Wv[d, n]; denominator via
                    # the ones-column matmul over P^T, folded into the copy-out
                    for qj in range(CHUNK // P):
                        o0 = psum.tile([P, F], f32, tag="av", bufs=4)
                        o1 = psum.tile([P, F], f32, tag="av", bufs=4)
                        rs = psum.tile([P, 1], f32, tag="rs", bufs=2)
                        for c in range(CD):
                            st_, sp_ = (c == 0), (c == CD - 1)
                            lh = UT[:, c, qj * P:(qj + 1) * P]
                            nc.tensor.matmul(o0[:], lh, wv_s[:, c, 0:F],
                                             start=st_, stop=sp_)
                            nc.tensor.matmul(o1[:], lh, wv_s[:, c, F:2 * F],
                                             start=st_, stop=sp_)
                        for ki in range(nk):
                            nc.tensor.matmul(rs[:], PT[:, ki, qj * P:(qj + 1) * P],
                                             ones[:, 0:1],
                                             start=(ki == 0), stop=(ki == nk - 1))
                        rcp = work.tile([P, 1], f32, tag="rcp", bufs=4)
                        nc.vector.reciprocal(rcp[:], rs[:])
                        ot = work.tile([P, D], bf, tag="ot", bufs=4)
                        nc.vector.tensor_scalar_mul(ot[:, 0:F], o0[:], rcp[:])
                        nc.vector.tensor_scalar_mul(ot[:, F:2 * F], o1[:], rcp[:])
                        row = qb + qj * P
                        nc.sync.dma_start(out=out_d[row:row + P, :], in_=ot[:])
    nc.finalize()
    return nc


_NC_CACHE = {}


def _get_nc(reps: int = 1):
    if reps not in _NC_CACHE:
        _NC_CACHE[reps] = build_nc(reps)
    return _NC_CACHE[reps]


def _masks():
    """Additive bf16 masks per half: 0 where k <= global q position, else -1e30."""
    q = np.arange(CHUNK)[None, :]
    out = []
    for h in range(2):
        ms = []
        for ci in range(len(KV)):
            k = np.arange(KV[ci])[:, None]
            ms.append(np.where(k <= q + QROWS[h][ci], 0.0, NEG).astype(BF16))
        out.append(ms)
    return out


def make_in_maps(x, Wq, Wk, Wv):
    M = np.asarray(Wq, np.float32) @ np.asarray(Wk, np.float32).T
    wqkb = np.ascontiguousarray(M.astype(BF16))
    wvb = np.ascontiguousarray(np.asarray(Wv).astype(BF16))
    masks = _masks()
    in_maps = []
    for i in range(8):
        b, h = i // 2, i % 2
        xb = x[b].astype(BF16)
        xT = np.ascontiguousarray(xb.T)
        xq = np.concatenate([xb[r:r + CHUNK] for r in QROWS[h]], axis=0)
        xqT = np.ascontiguousarray(xq.T)
        m = {
            "xkv": xT, "xun": np.ascontiguousarray(xb), "xq": xqT,
            "wqk": wqkb, "wv": wvb,
        }
        for ci in range(len(KV)):
            m[f"mask{ci}"] = masks[h][ci]
        in_maps.append(m)
    return in_maps


def gather_out(results, x_dtype=np.float32):
    out = np.empty((B, S, D), x_dtype)
    for i in range(8):
        b, h = i // 2, i % 2
        o = np.asarray(results[i]["out"]).astype(x_dtype)
        for ci, r in enumerate(QROWS[h]):
            out[b, r:r + CHUNK] = o[ci * CHUNK:(ci + 1) * CHUNK]
    return out


def run_cores(in_maps, **kwargs):
    return run_bass_kernel_spmd(_get_nc(), in_maps, core_ids=list(range(8)), **kwargs)


def kernel(x, Wq, Wk, Wv):
    x = np.asarray(x)
    in_maps = make_in_maps(x, np.asarray(Wq), np.asarray(Wk), np.asarray(Wv))
    res = run_cores(in_maps)
    return gather_out(res.results)


# revision 8
# speedup vs baseline: 1.1046x; 1.0872x over previous
"""Causal single-head attention (B=4, S=2048, d=1024, f32) on 8 TRN2 NeuronCores.

v4: fully factorized, collective-free. Core i = (b = i//2, h = i%2) with the
zig-zag 256-row query-block assignment of v1-v3.

Both weight projections are fused away algebraically:
  scores = (x Wq)(x Wk)^T = x (Wq Wk^T) x^T   -> host passes M = Wq Wk^T,
      the kernel computes Q' = x_q M (qf-outer) and contracts scores against
      the resident x^T tile (stationary = xkv itself; no K tensor exists).
  out = P V = (P x) Wv                        -> per chunk the kernel computes
      U^T = x^T-contracted P (stationary = untransposed x tiles, moving = P^T)
      and then out = U Wv with the full Wv resident in SBUF.
Total matmul cycles equal v3's, but there is NO V projection, NO pairwise
exchange, and NO collectives — every core works standalone on its (batch,
query-half), which removes the cross-core rendezvous from the critical path.

The denominator is unchanged: per (chunk, qj) a ones-column matmul accumulates
sum_k P^T[k, q] into PSUM alongside; the division is folded into the final
PSUM->SBUF copy (per-partition tensor_scalar_mul), valid because out[q, n]
lands q-partitioned exactly as in v3.

x is needed in both layouts (x^T for scores / Q', x for the P-contraction), so
the host passes xkv [d, S] and xun [S, d]; both are single-buffered — their
next-rep loads start after this rep's last stationary read, which leaves
enough slack to hide the 4MB reloads under the following rep's early phases.

bf16 compute with f32 PSUM accumulation, DMA blessing via in-place DVE copies,
host-precomputed additive causal masks — all as in v1-v3.
"""

import numpy as np
import ml_dtypes

import concourse.bass as bass
from concourse import bacc
import concourse.mybir as mybir
from concourse.tile import TileContext
from concourse.bass_utils import run_bass_kernel_spmd

P = 128
B = 4
S = 2048          # sequence length (= keys per batch)
D = 1024          # d_in = d_out
HALF = 1024       # queries per core
CHUNK = 256       # query chunk
CD = D // P       # 8 contraction tiles
SK = S // P       # 16 key tiles
F = 512           # matmul moving free dim (one PSUM bank of f32)
KV = (512, 1024, 1536, 2048)   # kv prefix length per chunk
QBASE = (0, 256, 512, 768)     # local query offset of each chunk
SCALE = 1.0 / 32.0    # 1/sqrt(d_k)
NEG = -1.0e30         # additive mask for disallowed (k, q)

QROWS = ((0, 512, 1280, 1792), (256, 768, 1024, 1536))

BF16 = ml_dtypes.bfloat16


def build_nc(reps: int = 1) -> bacc.Bacc:
    nc = bacc.Bacc("TRN2")
    bf = mybir.dt.bfloat16
    f32 = mybir.dt.float32

    xkv_d = nc.declare_dram_parameter("xkv", [D, S], bf, isOutput=False)
    xun_d = nc.declare_dram_parameter("xun", [S, D], bf, isOutput=False)
    xq_d = nc.declare_dram_parameter("xq", [D, HALF], bf, isOutput=False)
    wqk_d = nc.declare_dram_parameter("wqk", [D, D], bf, isOutput=False)
    wv_d = nc.declare_dram_parameter("wv", [D, D], bf, isOutput=False)
    m_d = [
        nc.declare_dram_parameter(f"mask{ci}", [KV[ci], CHUNK], bf, isOutput=False)
        for ci in range(len(KV))
    ]
    out_d = nc.declare_dram_parameter("out", [HALF, D], bf, isOutput=True)

    with TileContext(nc) as tc:
        with tc.tile_pool(name="persist", bufs=1) as persist, \
             tc.tile_pool(name="work", bufs=1) as work, \
             tc.tile_pool(name="psum", bufs=1, space="PSUM") as psum:
            QT = persist.tile([P, CD, HALF], bf)
            ones = persist.tile([P, 1], bf)
            nc.vector.memset(ones[:], 1.0)

            def load(dst, dram_t, c):
                nc.sync.dma_start(out=dst[:, c], in_=dram_t[c * P:(c + 1) * P, :])
                nc.vector.tensor_copy(dst[:, c], dst[:, c])

            for _rep in range(reps):
                xq_s = work.tile([P, CD, HALF], bf, tag="xq")
                wqk_s = work.tile([P, CD, D], bf, tag="wqk")
                xkv_s = work.tile([P, CD, S], bf, tag="big")
                xun_s = work.tile([P, SK, D], bf, tag="xun")
                wv_s = work.tile([P, CD, D], bf, tag="wv", bufs=2)
                # xq/wqk first: Q' proj is the first PE work of the rep
                for c in range(CD):
                    load(xq_s, xq_d, c)
                    load(wqk_s, wqk_d, c)
                for c in range(CD):
                    load(xkv_s, xkv_d, c)
                    load(wv_s, wv_d, c)
                for c in range(SK):
                    load(xun_s, xun_d, c)

                # ---------------- Q'^T = M^T xq^T (qf-outer) ----------------
                for qf in range(HALF // F):
                    for m in range(CD):
                        ps = psum.tile([P, F], f32, tag="pp", bufs=3)
                        for c in range(CD):
                            nc.tensor.matmul(
                                ps[:],
                                wqk_s[:, c, m * P:(m + 1) * P],
                                xq_s[:, c, qf * F:(qf + 1) * F],
                                start=(c == 0), stop=(c == CD - 1),
                            )
                        nc.vector.tensor_copy(QT[:, m, qf * F:(qf + 1) * F], ps[:])

                # ---------------- attention ----------------
                for ci in range(len(KV)):
                    nk = KV[ci] // P
                    qb = QBASE[ci]
                    md = m_d[ci]
                    PT = work.tile([P, SK, CHUNK], bf, tag="pt")
                    vmin = min(QROWS[0][ci], QROWS[1][ci])
                    # scores^T[k, q] = sum_c x^T[c, k]^T Q'^T[c, q]
                    for ki in range(nk):
                        masked = ki * P + P - 1 > vmin
                        if masked:
                            mt = work.tile([P, CHUNK], bf, tag="mask", bufs=4)
                            nc.sync.dma_start(
                                out=mt[:], in_=md[ki * P:(ki + 1) * P, :])
                            nc.vector.tensor_copy(mt[:], mt[:])
                        ps = psum.tile([P, CHUNK], f32, tag="pp", bufs=3)
                        for c in range(CD):
                            nc.tensor.matmul(
                                ps[:],
                                xkv_s[:, c, ki * P:(ki + 1) * P],
                                QT[:, c, qb:qb + CHUNK],
                                start=(c == 0), stop=(c == CD - 1),
                            )
                        if masked:
                            nc.vector.tensor_add(ps[:], ps[:], mt[:])
                        pe = work.tile([P, CHUNK], bf, tag="pexp", bufs=2)
                        nc.scalar.activation(
                            pe[:], ps[:],
                            mybir.ActivationFunctionType.Exp, scale=SCALE,
                        )
                        nc.vector.tensor_copy(PT[:, ki], pe[:])
                    # U^T[d, q] = sum_k x[k, d]^T P^T[k, q]  (both qj at once)
                    UT = work.tile([P, CD, CHUNK], bf, tag="ut", bufs=2)
                    for db in range(CD):
                        pu = psum.tile([P, CHUNK], f32, tag="pp", bufs=3)
                        for ki in range(nk):
                            nc.tensor.matmul(
                                pu[:],
                                xun_s[:, ki, db * P:(db + 1) * P],
                                PT[:, ki, 0:CHUNK],
                                start=(ki == 0), stop=(ki == nk - 1),
                            )
                        nc.vector.tensor_copy(UT[:, db, :], pu[:])
                    # out[q, n] = sum_d U^T[d, q]^T Wv[d, n]; denominator via
                    # the ones-column matmul over P^T, folded into the copy-out
                    for qj in range(CHUNK // P):
                        o0 = psum.tile([P, F], f32, tag="av", bufs=3)
                        o1 = psum.tile([P, F], f32, tag="av", bufs=3)
                        rs = psum.tile([P, 1], f32, tag="rs", bufs=2)
                        for c in range(CD):
                            st_, sp_ = (c == 0), (c == CD - 1)
                            lh = UT[:, c, qj * P:(qj + 1) * P]
                            nc.tensor.matmul(o0[:], lh, wv_s[:, c, 0:F],
                                             start=st_, stop=sp_)
                            nc.tensor.matmul(o1[:], lh, wv_s[:, c, F:2 * F],
                                             start=st_, stop=sp_)
                        for ki in range(nk):
                            nc.tensor.matmul(rs[:], PT[:, ki, qj * P:(qj + 1) * P],
                                             ones[:, 0:1],
                                             start=(ki == 0), stop=(ki == nk - 1))
                        rcp = work.tile([P, 1], f32, tag="rcp", bufs=4)
                        nc.vector.reciprocal(rcp[:], rs[:])
                        ot = work.tile([P, D], bf, tag="ot", bufs=4)
                        nc.vector.tensor_scalar_mul(ot[:, 0:F], o0[:], rcp[:])
                        nc.vector.tensor_scalar_mul(ot[:, F:2 * F], o1[:], rcp[:])
                        row = qb + qj * P
                        nc.sync.dma_start(out=out_d[row:row + P, :], in_=ot[:])
    nc.finalize()
    return nc


_NC_CACHE = {}


def _get_nc(reps: int = 1):
    if reps not in _NC_CACHE:
        _NC_CACHE[reps] = build_nc(reps)
    return _NC_CACHE[reps]


def _masks():
    """Additive bf16 masks per half: 0 where k <= global q position, else -1e30."""
    q = np.arange(CHUNK)[None, :]
    out = []
    for h in range(2):
        ms = []
        for ci in range(len(KV)):
            k = np.arange(KV[ci])[:, None]
            ms.append(np.where(k <= q + QROWS[h][ci], 0.0, NEG).astype(BF16))
        out.append(ms)
    return out


def make_in_maps(x, Wq, Wk, Wv):
    M = np.asarray(Wq, np.float32) @ np.asarray(Wk, np.float32).T
    wqkb = np.ascontiguousarray(M.astype(BF16))
    wvb = np.ascontiguousarray(np.asarray(Wv).astype(BF16))
    masks = _masks()
    in_maps = []
    for i in range(8):
        b, h = i // 2, i % 2
        xb = x[b].astype(BF16)
        xT = np.ascontiguousarray(xb.T)
        xq = np.concatenate([xb[r:r + CHUNK] for r in QROWS[h]], axis=0)
        xqT = np.ascontiguousarray(xq.T)
        m = {
            "xkv": xT, "xun": np.ascontiguousarray(xb), "xq": xqT,
            "wqk": wqkb, "wv": wvb,
        }
        for ci in range(len(KV)):
            m[f"mask{ci}"] = masks[h][ci]
        in_maps.append(m)
    return in_maps


def gather_out(results, x_dtype=np.float32):
    out = np.empty((B, S, D), x_dtype)
    for i in range(8):
        b, h = i // 2, i % 2
        o = np.asarray(results[i]["out"]).astype(x_dtype)
        for ci, r in enumerate(QROWS[h]):
            out[b, r:r + CHUNK] = o[ci * CHUNK:(ci + 1) * CHUNK]
    return out


def run_cores(in_maps, **kwargs):
    return run_bass_kernel_spmd(_get_nc(), in_maps, core_ids=list(range(8)), **kwargs)


def kernel(x, Wq, Wk, Wv):
    x = np.asarray(x)
    in_maps = make_in_maps(x, np.asarray(Wq), np.asarray(Wk), np.asarray(Wv))
    res = run_cores(in_maps)
    return gather_out(res.results)
